# revision 2
# baseline (speedup 1.0000x reference)
"""CorrectedLinear on 8 TRN2 NeuronCores.

Math: out = x @ W.T + b + (x @ V_r) @ C.T
    = x @ (W.T + V_r @ C.T) + b          -- fold the rank-32 correction
      into a single effective weight matrix Wt [d_in, d_out] (0.05% of
      the GEMM FLOPs, done host-side in float64).

Sharding: pure data-parallel over the batch dim (8 batches -> 8 cores).
Each core computes a [8192, 1024] x [1024, 1024] GEMM.

Device layout: the PE contracts along the partition axis, so both
operands need d_in on partitions. x is fed pre-transposed per core
(xT [d_in, t]) and the output is produced transposed (outT [d_out, t]),
un-transposed on the host. All matmul operands use float32r (full-rate
fp32, ~1e-4 matmul precision) with fp32 PSUM accumulation.
"""

import numpy as np

N_CORES = 8
T = 8192          # tokens per core (batch entry)
D = 1024          # d_in
O = 1024          # d_out
TCH = 512         # moving free dim per matmul (= one PSUM bank of fp32)
NT = T // TCH     # 16 t-chunks
ND = D // 128     # 8 contraction slices
NO = O // 128     # 8 output-partition slices

_nc = None


def _build():
    import concourse.bacc as bacc
    import concourse.mybir as mybir
    import concourse.tile as tile

    f32 = mybir.dt.float32
    f32r = mybir.dt.float32r

    nc = bacc.Bacc(
        "TRN2", target_bir_lowering=False, debug=False, num_devices=N_CORES
    )
    xT_d = nc.dram_tensor("xT", [D, T], f32r, kind="ExternalInput")
    Wt_d = nc.dram_tensor("Wt", [D, O], f32r, kind="ExternalInput")
    b_d = nc.dram_tensor("bb", [O], f32, kind="ExternalInput")
    outT_d = nc.dram_tensor("outT", [O, T], f32, kind="ExternalOutput")

    with tile.TileContext(nc) as tc:
        with (
            tc.tile_pool(name="wt", bufs=1) as wt_pool,
            tc.tile_pool(name="bp", bufs=1) as b_pool,
            tc.tile_pool(name="xp", bufs=2 * ND) as x_pool,
            tc.tile_pool(name="op", bufs=4) as o_pool,
            tc.tile_pool(name="ps", bufs=4, space="PSUM") as psum_pool,
        ):
            b_sb = b_pool.tile([128, NO], f32, name="b_sb")
            nc.sync.dma_start(
                out=b_sb[:], in_=b_d.ap().rearrange("(j p) -> p j", p=128)
            )
            wts = []
            for d in range(ND):
                w = wt_pool.tile([128, O], f32r, name=f"w{d}", tag=f"w{d}")
                nc.sync.dma_start(out=w[:], in_=Wt_d.ap()[d * 128 : (d + 1) * 128, :])
                wts.append(w)

            for t in range(NT):
                xs = []
                for d in range(ND):
                    xt = x_pool.tile([128, TCH], f32r, name="xt", tag="xt")
                    nc.sync.dma_start(
                        out=xt[:],
                        in_=xT_d.ap()[
                            d * 128 : (d + 1) * 128, t * TCH : (t + 1) * TCH
                        ],
                    )
                    xs.append(xt)
                for o in range(NO):
                    acc = psum_pool.tile([128, TCH], f32, name="acc", tag="acc")
                    for d in range(ND):
                        nc.tensor.matmul(
                            acc[:],
                            wts[d][:, o * 128 : (o + 1) * 128],
                            xs[d][:],
                            start=(d == 0),
                            stop=(d == ND - 1),
                        )
                    ot = o_pool.tile([128, TCH], f32, name="ot", tag="ot")
                    nc.scalar.activation(
                        ot[:],
                        acc[:],
                        mybir.ActivationFunctionType.Identity,
                        bias=b_sb[:, o : o + 1],
                    )
                    nc.scalar.dma_start(
                        out=outT_d.ap()[
                            o * 128 : (o + 1) * 128, t * TCH : (t + 1) * TCH
                        ],
                        in_=ot[:],
                    )
    nc.compile()
    return nc


def _get_nc():
    global _nc
    if _nc is None:
        _nc = _build()
    return _nc


def _make_in_maps(x, W, b, V_r, C):
    Wt = (
        W.astype(np.float64).T + V_r.astype(np.float64) @ C.astype(np.float64).T
    ).astype(np.float32)
    b = np.ascontiguousarray(b, dtype=np.float32)
    return [
        {
            "xT": np.ascontiguousarray(x[i].T.astype(np.float32, copy=False)),
            "Wt": Wt,
            "bb": b,
        }
        for i in range(N_CORES)
    ]


def _execute(in_maps, trace=False):
    from concourse.bass_utils import run_bass_kernel_spmd

    return run_bass_kernel_spmd(
        _get_nc(), in_maps, list(range(N_CORES)), trace=trace
    )


def kernel(x, W, b, V_r, C):
    res = _execute(_make_in_maps(x, W, b, V_r, C))
    out = np.empty((N_CORES, T, O), dtype=np.float32)
    for i in range(N_CORES):
        out[i] = res.results[i]["outT"].T
    return out
